# revision 1
# baseline (speedup 1.0000x reference)
"""AttentiveGraphConvolution (GAT-style layer) on 8 trn2 NeuronCores.

Math (reference):
    h   = x @ W                       [N, D]
    a_s = h @ attn_self               [N, 1]
    a_n = h @ attn_neigh              [N, 1]
    e   = leaky_relu(a_s + a_n.T, 0.2)
    e   = e + NEG_INF * (1 - adj)
    out = relu(softmax(e, -1) @ h)

Reformulation used here (exact in fp32 up to rounding):
    s_ij = a_s[i] + a_n[j]
    exp(leaky(s)) = exp(0.2 s) * max(exp(0.8 s), 1)       (leaky alpha = 0.2)
    exp(0.8 s)    = w[i] * w2[j],  w = e^{0.8 a_s}, w2 = e^{0.8 a_n}
    adj binary =>  masked weight t_ij = adj_ij * u2_i * v2_j * max(w_i w2_j, 1)

    out_i = relu( (sum_j t_ij h_j) / (sum_j t_ij) )
          = relu( (sum_j q_ji h2_j) / (sum_j q_ji v2_j) )   (u2_i cancels)
    with q_ji  = adjT_ji * max(w_i w2_j, 1)                 [j, i] layout
         h2_j  = v2_j * h_j

Per adj tile the device work is:  R = w2_j * W_bcast  (ACT copy-with-scale),
q = (R max 1) * adjT  (DVE scalar_tensor_tensor), then accumulating float32r
matmuls  outT += h2_chunk.T @ q  and  rs += v2_chunk.T @ q  on the PE.

Sharding: output rows across 8 cores. Each core receives its adj row-slab as
bf16 (adj is binary so bf16 is exact), pre-transposed and row-interleaved in
groups of GP=4 (host layout choice) so each DMA descriptor covers 4 adjacency
rows = 8 KB contiguous.  x is sharded; h2 shards are computed locally,
written in a partition-major layout, and AllGathered (~4 MB) through DRAM so
the read-back also gets 4 KB descriptors.
"""

import numpy as np

N = 8192
DIN = 512
DOUT = 128
NCORES = 8
S = N // NCORES     # 1024 output rows per core
GP = 4              # adjacency rows per partition per DMA (descriptor size)


def _emit(nc, tc, ctx, n, s, din, dout):
    from concourse import masks, mybir

    f32 = mybir.dt.float32
    f32r = mybir.dt.float32r
    bf16 = mybir.dt.bfloat16
    AF = mybir.ActivationFunctionType
    ALU = mybir.AluOpType

    P = 128
    jc_n = n // P       # j chunks over all nodes
    sc_n = s // P       # chunks in the local row slab
    kc_n = din // P     # contraction chunks for x @ W
    nb = min(512, s)    # matmul moving-dim block
    ib_n = s // nb      # i blocks per core (free dim of main matmuls)
    g_n = jc_n // GP    # adj super-chunks (GP j-chunks per DMA)

    adjt = nc.dram_tensor("adjt", [n, s], bf16, kind="ExternalInput")
    xt = nc.dram_tensor("xt", [din, s], f32r, kind="ExternalInput")
    wmat = nc.dram_tensor("wmat", [din, dout], f32r, kind="ExternalInput")
    att = nc.dram_tensor("att", [dout, 2], f32, kind="ExternalInput")
    out = nc.dram_tensor("out", [s, dout], f32, kind="ExternalOutput")

    const_pool = ctx.enter_context(tc.tile_pool(name="const", bufs=1))
    ph1_pool = ctx.enter_context(tc.tile_pool(name="ph1", bufs=1))
    ph1_psum = ctx.enter_context(tc.tile_pool(name="ph1_psum", bufs=1, space="PSUM"))
    tp_psum = ctx.enter_context(tc.tile_pool(name="tp_psum", bufs=2, space="PSUM"))
    acc_psum = ctx.enter_context(tc.tile_pool(name="acc_psum", bufs=1, space="PSUM"))
    dram_pool = ctx.enter_context(tc.tile_pool(name="dram", bufs=1, space="DRAM"))
    adj_pool = ctx.enter_context(tc.tile_pool(name="adj", bufs=6))
    r_pool = ctx.enter_context(tc.tile_pool(name="r", bufs=5))
    q_pool = ctx.enter_context(tc.tile_pool(name="q", bufs=8))
    fin_pool = ctx.enter_context(tc.tile_pool(name="fin", bufs=2))

    ident = const_pool.tile([P, P], f32, name="ident")
    masks.make_identity(nc, ident[:])

    # ---- Phase 1: local h shard, attention logit vectors -------------------
    w_sb = []
    x_sb = []
    for k in range(kc_n):
        wt = ph1_pool.tile([P, P], f32r, name="w_sb", tag=f"w_sb{k}")
        nc.sync.dma_start(wt[:], wmat[k * P:(k + 1) * P, :])
        w_sb.append(wt)
        xt_t = ph1_pool.tile([P, s], f32r, name="x_sb", tag=f"x_sb{k}")
        nc.sync.dma_start(xt_t[:], xt[k * P:(k + 1) * P, :])
        x_sb.append(xt_t)
    att_sb = const_pool.tile([P, 2], f32, name="att_sb")
    nc.sync.dma_start(att_sb[:], att[:])

    # hT[d, n_local] = (x @ W).T for the local slab, built nb columns at a time
    hT_sb = ph1_pool.tile([P, s], f32, name="hT_sb")
    av_sb = ph1_pool.tile([2, s], f32, name="av_sb")  # rows: a_s, a_n (local)
    for b in range(s // nb):
        hT_ps = ph1_psum.tile([P, nb], f32, name="hT_ps")
        for k in range(kc_n):
            nc.tensor.matmul(
                hT_ps[:],
                w_sb[k][:],
                x_sb[k][:, b * nb:(b + 1) * nb],
                start=(k == 0),
                stop=(k == kc_n - 1),
            )
        nc.scalar.activation(hT_sb[:, b * nb:(b + 1) * nb], hT_ps[:], AF.Copy)
        av_ps = ph1_psum.tile([2, nb], f32, name="av_ps")
        nc.tensor.matmul(
            av_ps[:], att_sb[:], hT_sb[:, b * nb:(b + 1) * nb],
            start=True, stop=True,
        )
        nc.scalar.activation(av_sb[:, b * nb:(b + 1) * nb], av_ps[:], AF.Copy)

    # ---- Phase 2a: gather raw a_n early (small, unblocks the main loop) ----
    groups = [list(range(NCORES))]
    an_dram = dram_pool.tile([s, 1], f32, name="an_dram")
    nc.sync.dma_start(an_dram[:].rearrange("s o -> o s"), av_sb[1:2, :])
    anfull_dram = dram_pool.tile([n, 1], f32, addr_space="Shared", name="anfull")
    nc.gpsimd.collective_compute(
        "AllGather", ALU.bypass, replica_groups=groups,
        ins=[an_dram.opt()], outs=[anfull_dram.opt()],
    )
    anf_raw = ph1_pool.tile([jc_n, P], f32, name="anf_raw")
    nc.sync.dma_start(anf_raw[:], anfull_dram[:].rearrange("(k p) o -> k (p o)", p=P))
    anf_ps = tp_psum.tile([P, jc_n], f32, name="anf_ps", tag="tp")
    nc.tensor.matmul(anf_ps[:], anf_raw[:], ident[:jc_n, :jc_n],
                     is_transpose=True, start=True, stop=True)
    w2_sb = const_pool.tile([P, jc_n], f32, name="w2_sb")
    nc.scalar.activation(w2_sb[:], anf_ps[:], AF.Exp, scale=0.8)
    v2f_sb = const_pool.tile([P, jc_n], f32r, name="v2f_sb")
    nc.scalar.activation(v2f_sb[:], anf_ps[:], AF.Exp, scale=0.2)

    # W_bcast[p, i] = exp(0.8 * a_s_local[i]) for every partition p
    wrow_sb = ph1_pool.tile([1, s], f32, name="wrow_sb")
    nc.scalar.activation(wrow_sb[:], av_sb[0:1, :], AF.Exp, scale=0.8)
    ones_sb = const_pool.tile([1, P], f32, name="ones_sb")
    nc.gpsimd.memset(ones_sb[:], 1.0)
    wb_sb = const_pool.tile([P, s], f32, name="wb_sb")
    for b in range(s // nb):
        wb_ps = tp_psum.tile([P, nb], f32, name="wb_ps", tag="tp")
        nc.tensor.matmul(
            wb_ps[:], ones_sb[:], wrow_sb[:, b * nb:(b + 1) * nb],
            start=True, stop=True,
        )
        nc.scalar.activation(wb_sb[:, b * nb:(b + 1) * nb], wb_ps[:], AF.Copy)

    # ---- Phase 2b: h2 shard in partition-major layout, AllGather -----------
    # Local chunk c is written to rows {p*sc_n + c} so that the gathered
    # tensor reads back with 4 KB-contiguous per-partition descriptors.
    anT_sb = ph1_pool.tile([P, sc_n], f32, name="anT_sb")
    for c in range(sc_n):
        avT_ps = tp_psum.tile([P, 2], f32, name="avT_ps", tag="tp")
        nc.tensor.matmul(
            avT_ps[:], av_sb[:, c * P:(c + 1) * P], ident[:2, :2],
            is_transpose=True, start=True, stop=True,
        )
        nc.scalar.activation(anT_sb[:, c:c + 1], avT_ps[:, 1:2], AF.Copy)
    v2loc_sb = ph1_pool.tile([P, sc_n], f32, name="v2loc_sb")
    nc.scalar.activation(v2loc_sb[:], anT_sb[:], AF.Exp, scale=0.2)

    h2an_dram = dram_pool.tile([s, dout], f32r, name="h2an_dram")
    h2an_pm = h2an_dram[:].rearrange("(p kl) d -> kl p d", kl=sc_n)
    for c in range(sc_n):
        hn_ps = tp_psum.tile([P, P], f32, name="hn_ps", tag="tp")
        nc.tensor.matmul(
            hn_ps[:], hT_sb[:, c * P:(c + 1) * P], ident[:],
            is_transpose=True, start=True, stop=True,
        )
        h2c_sb = fin_pool.tile([P, dout], f32r, name="h2c_sb")
        nc.scalar.activation(h2c_sb[:], hn_ps[:], AF.Copy, scale=v2loc_sb[:, c:c + 1])
        nc.sync.dma_start(h2an_pm[c], h2c_sb[:])

    h2full_dram = dram_pool.tile([n, dout], f32r, addr_space="Shared", name="h2full")
    nc.gpsimd.collective_compute(
        "AllGather", ALU.bypass, replica_groups=groups,
        ins=[h2an_dram.opt()], outs=[h2full_dram.opt()],
    )

    # ---- Phase 3: load gathered h2 (4 KB descriptors per core block) -------
    h2big = ph1_pool.tile([P, jc_n * dout], f32r, name="h2big")
    for c in range(NCORES):
        nc.sync.dma_start(
            h2big[:, c * sc_n * dout:(c + 1) * sc_n * dout],
            h2full_dram[c * s:(c + 1) * s, :].rearrange(
                "(p kl) d -> p (kl d)", kl=sc_n),
        )

    # ---- Phase 4: main loop over adj super-chunks --------------------------
    mm_ps = [acc_psum.tile([P, nb], f32, name=f"mm_ps{b}") for b in range(ib_n)]
    rs_ps = [acc_psum.tile([1, nb], f32, name=f"rs_ps{b}") for b in range(ib_n)]
    for g in range(g_n):
        adj_t = adj_pool.tile([P, GP * s], bf16, name="adj_t")
        nc.sync.dma_start(
            adj_t[:],
            adjt[g * GP * P:(g + 1) * GP * P, :].rearrange(
                "(p r) i -> p (r i)", r=GP),
        )
        for r in range(GP):
            j = g * GP + r
            r_t = r_pool.tile([P, s], f32, name="r_t")
            nc.scalar.activation(r_t[:], wb_sb[:], AF.Copy, scale=w2_sb[:, j:j + 1])
            q_t = q_pool.tile([P, s], f32r, name="q_t")
            nc.vector.scalar_tensor_tensor(
                q_t[:], r_t[:], 1.0, adj_t[:, r * s:(r + 1) * s],
                op0=ALU.max, op1=ALU.mult,
            )
            for b in range(ib_n):
                nc.tensor.matmul(
                    mm_ps[b][:], h2big[:, j * dout:(j + 1) * dout],
                    q_t[:, b * nb:(b + 1) * nb],
                    start=(j == 0), stop=(j == jc_n - 1),
                )
            for b in range(ib_n):
                nc.tensor.matmul(
                    rs_ps[b][:], v2f_sb[:, j:j + 1], q_t[:, b * nb:(b + 1) * nb],
                    start=(j == 0), stop=(j == jc_n - 1),
                )

    # ---- Phase 5: normalize, relu, transpose out ---------------------------
    rs_sb = ph1_pool.tile([1, s], f32, name="rs_sb")
    for b in range(ib_n):
        nc.scalar.activation(rs_sb[:, b * nb:(b + 1) * nb], rs_ps[b][:], AF.Copy)
    rs_dram = dram_pool.tile([sc_n, P], f32, name="rs_dram")
    nc.sync.dma_start(rs_dram[:].rearrange("k p -> (k p)")[None, :], rs_sb[0:1, :])
    rs_raw = ph1_pool.tile([sc_n, P], f32, name="rs_raw")
    nc.sync.dma_start(rs_raw[:], rs_dram[:])
    rsT_ps = tp_psum.tile([P, sc_n], f32, name="rsT_ps", tag="tp")
    nc.tensor.matmul(rsT_ps[:], rs_raw[:], ident[:sc_n, :sc_n],
                     is_transpose=True, start=True, stop=True)
    rrT_sb = ph1_pool.tile([P, sc_n], f32, name="rrT_sb")
    nc.vector.reciprocal(rrT_sb[:], rsT_ps[:])

    mo_sb = ph1_pool.tile([P, s], f32, name="mo_sb")
    for b in range(ib_n):
        nc.scalar.activation(mo_sb[:, b * nb:(b + 1) * nb], mm_ps[b][:], AF.Copy)
    for c in range(sc_n):
        ot_ps = tp_psum.tile([P, P], f32, name="ot_ps", tag="tp")
        nc.tensor.matmul(
            ot_ps[:], mo_sb[:, c * P:(c + 1) * P], ident[:],
            is_transpose=True, start=True, stop=True,
        )
        oc_sb = fin_pool.tile([P, dout], f32, name="oc_sb")
        nc.scalar.activation(oc_sb[:], ot_ps[:], AF.Relu, scale=rrT_sb[:, c:c + 1])
        nc.sync.dma_start(out[c * P:(c + 1) * P, :], oc_sb[:])


def build_nc(n=N, s=S, din=DIN, dout=DOUT):
    from contextlib import ExitStack

    import concourse.bacc as bacc
    import concourse.tile as tile

    nc = bacc.Bacc(
        "TRN2",
        target_bir_lowering=False,
        debug=False,
        num_devices=NCORES,
    )
    with tile.TileContext(nc) as tc, ExitStack() as ctx:
        _emit(nc, tc, ctx, n, s, din, dout)
    nc.compile()
    return nc


def prep_adjt(adj_slab):
    """[s, n] adj row-slab -> transposed [n, s] bf16 with GP-row interleave."""
    import ml_dtypes

    adjt = adj_slab.T  # [n, s]
    n, s = adjt.shape
    P = 128
    g = n // (GP * P)
    adjt = adjt.reshape(g, GP, P, s).transpose(0, 2, 1, 3).reshape(n, s)
    return np.ascontiguousarray(adjt.astype(ml_dtypes.bfloat16))


def make_in_maps(x, adj, W, attn_self, attn_neigh, s=S):
    att = np.concatenate([attn_self, attn_neigh], axis=1).astype(np.float32)
    in_maps = []
    for c in range(NCORES):
        sl = slice(c * s, (c + 1) * s)
        in_maps.append({
            "adjt": prep_adjt(adj[sl, :]),
            "xt": np.ascontiguousarray(x[sl, :].T),
            "wmat": np.ascontiguousarray(W),
            "att": att,
        })
    return in_maps


def kernel(x, adj, W, attn_self, attn_neigh):
    from concourse.bass_utils import run_bass_kernel_spmd

    x = np.asarray(x, dtype=np.float32)
    adj = np.asarray(adj, dtype=np.float32)
    W = np.asarray(W, dtype=np.float32)
    attn_self = np.asarray(attn_self, dtype=np.float32)
    attn_neigh = np.asarray(attn_neigh, dtype=np.float32)

    nc = build_nc()
    in_maps = make_in_maps(x, adj, W, attn_self, attn_neigh)
    res = run_bass_kernel_spmd(nc, in_maps, list(range(NCORES)))
    return np.concatenate([res.results[c]["out"] for c in range(NCORES)], axis=0)



# revision 6
# speedup vs baseline: 1.1554x; 1.1554x over previous
"""AttentiveGraphConvolution (GAT-style layer) on 8 trn2 NeuronCores.

Math (reference):
    h   = x @ W                       [N, D]
    a_s = h @ attn_self               [N, 1]
    a_n = h @ attn_neigh              [N, 1]
    e   = leaky_relu(a_s + a_n.T, 0.2)
    e   = e + NEG_INF * (1 - adj)
    out = relu(softmax(e, -1) @ h)

Reformulation (exact in exact arithmetic):
    exp(leaky(s)) = max(exp(s), exp(0.2 s)),  s_ij = a_s_i + a_n_j.
    Divide numerator and denominator of the softmax by exp(0.2*a_s_i) (>0,
    constant per output row) -- the ratio is unchanged:
        t_ij = A_ij * max(w_i*u_j, v2_j)
             = A_ij * v2_j * w_i * max(w2_j, invw_i)
    with w_i = e^{0.8 a_s_i}, invw_i = 1/w_i, w2_j = e^{0.8 a_n_j},
    v2_j = e^{0.2 a_n_j}.  The per-row factor w_i also cancels, so with
        q_ji   = adjT_ji * max(w2_j, invw_i)          [j, i] layout
        h2_j   = v2_j * h_j
    we get  out_i = relu( (sum_j q_ji h2_j) / (sum_j q_ji v2_j) ).

Device main loop per 128-node j-chunk (64 chunks):
    q   = (invw_bcast MAX w2col) * adjT_chunk   -- one DVE STT, all bf16
    numT += h2_chunk.T @ q                      -- PE bf16, 1024 moving rows
    den  += v2_chunk.T @ q                      -- PE bf16, 1024 moving rows

Sharding: output rows across 8 cores.  Each core receives its adj row-slab
pre-transposed as bf16 (binary, exact), split into 16 blocks of 4 chunks,
row-interleaved per block so each DMA descriptor covers 4 adjacency rows.
h2 (bf16) plus a_n (split hi/lo into two bf16 columns for f32 accuracy) are
AllGathered in two halves so the PE can start on the first half while the
second is still in flight.
"""

import numpy as np

N = 8192
DIN = 512
DOUT = 128
NCORES = 8
S = N // NCORES     # 1024 output rows per core
GP = 4              # adjacency rows per partition per DMA block
DH = DOUT + 2       # gathered row payload: h2 (128) + a_n hi + a_n lo


def _emit(nc, tc, ctx, n, s, din, dout):
    from concourse import masks, mybir

    f32 = mybir.dt.float32
    f32r = mybir.dt.float32r
    bf16 = mybir.dt.bfloat16
    AF = mybir.ActivationFunctionType
    ALU = mybir.AluOpType

    P = 128
    jc_n = n // P           # 64 j-chunks over all nodes
    sc_n = s // P           # 8 chunks in the local row slab
    kc_n = din // P         # 4 contraction chunks for x @ W
    nb = 512                # matmul moving-dim block (PSUM bank limit)
    ib_n = s // nb          # i blocks per core
    half_c = jc_n // 2      # 32 chunks per gather half
    hb = 4                  # local chunks per gather half buffer

    adjt = nc.dram_tensor("adjt", [n, s], bf16, kind="ExternalInput")
    xt = nc.dram_tensor("xt", [din, s], f32r, kind="ExternalInput")
    wmat = nc.dram_tensor("wmat", [din, dout], f32r, kind="ExternalInput")
    att = nc.dram_tensor("att", [dout, 2], f32, kind="ExternalInput")
    out = nc.dram_tensor("out", [s, dout], f32, kind="ExternalOutput")

    const_pool = ctx.enter_context(tc.tile_pool(name="const", bufs=1))
    ph1_pool = ctx.enter_context(tc.tile_pool(name="ph1", bufs=1))
    ph1_psum = ctx.enter_context(tc.tile_pool(name="ph1_psum", bufs=1, space="PSUM"))
    tp_psum = ctx.enter_context(tc.tile_pool(name="tp_psum", bufs=2, space="PSUM"))
    acc_psum = ctx.enter_context(tc.tile_pool(name="acc_psum", bufs=1, space="PSUM"))
    dram_pool = ctx.enter_context(tc.tile_pool(name="dram", bufs=1, space="DRAM"))
    adj_pool = ctx.enter_context(tc.tile_pool(name="adj", bufs=5))
    q_pool = ctx.enter_context(tc.tile_pool(name="q", bufs=6))
    fin_pool = ctx.enter_context(tc.tile_pool(name="fin", bufs=2))

    ident = const_pool.tile([P, P], f32, name="ident")
    masks.make_identity(nc, ident[:])

    # ---- Phase 1: local h shard, attention logit vectors -------------------
    w_sb = []
    x_sb = []
    for k in range(kc_n):
        wt = ph1_pool.tile([P, P], f32r, name="w_sb", tag=f"w_sb{k}")
        nc.sync.dma_start(wt[:], wmat[k * P:(k + 1) * P, :])
        w_sb.append(wt)
        xt_t = ph1_pool.tile([P, s], f32r, name="x_sb", tag=f"x_sb{k}")
        nc.sync.dma_start(xt_t[:], xt[k * P:(k + 1) * P, :])
        x_sb.append(xt_t)
    att_sb = const_pool.tile([P, 2], f32, name="att_sb")
    nc.sync.dma_start(att_sb[:], att[:])

    # hT[d, i_local] = (x @ W).T for the local slab
    hT_sb = ph1_pool.tile([P, s], f32, name="hT_sb")
    av_sb = ph1_pool.tile([2, s], f32, name="av_sb")  # rows: a_s, a_n (local)
    nb1 = 512
    for b in range(s // nb1):
        hT_ps = ph1_psum.tile([P, nb1], f32, name="hT_ps")
        for k in range(kc_n):
            nc.tensor.matmul(
                hT_ps[:],
                w_sb[k][:],
                x_sb[k][:, b * nb1:(b + 1) * nb1],
                start=(k == 0),
                stop=(k == kc_n - 1),
            )
        nc.scalar.activation(hT_sb[:, b * nb1:(b + 1) * nb1], hT_ps[:], AF.Copy)
        av_ps = ph1_psum.tile([2, nb1], f32, name="av_ps")
        nc.tensor.matmul(
            av_ps[:], att_sb[:], hT_sb[:, b * nb1:(b + 1) * nb1],
            start=True, stop=True,
        )
        nc.scalar.activation(av_sb[:, b * nb1:(b + 1) * nb1], av_ps[:], AF.Copy)

    # ---- Phase 2: local a_n transpose, h2 shard, gather (two halves) -------
    groups = [list(range(NCORES))]

    # anT[p, cl] = a_n_local[cl*128 + p]
    anT_sb = ph1_pool.tile([P, sc_n], f32, name="anT_sb")
    for c in range(sc_n):
        avT_ps = tp_psum.tile([P, 2], f32, name="avT_ps", tag="tp")
        nc.tensor.matmul(
            avT_ps[:], av_sb[:, c * P:(c + 1) * P], ident[:2, :2],
            is_transpose=True, start=True, stop=True,
        )
        nc.scalar.activation(anT_sb[:, c:c + 1], avT_ps[:, 1:2], AF.Copy)
    v2loc_sb = ph1_pool.tile([P, sc_n], f32, name="v2loc_sb")
    nc.scalar.activation(v2loc_sb[:], anT_sb[:], AF.Exp, scale=0.2)
    anhi_sb = ph1_pool.tile([P, sc_n], bf16, name="anhi_sb")
    nc.scalar.activation(anhi_sb[:], anT_sb[:], AF.Copy)
    anlo_sb = ph1_pool.tile([P, sc_n], bf16, name="anlo_sb")
    nc.vector.tensor_tensor(anlo_sb[:], anT_sb[:], anhi_sb[:], ALU.subtract)

    # per-half gather buffers, pm-interleaved so read-back descriptors are
    # hb*DH*2 = 1040 B contiguous per partition
    h2half_dram = []
    h2full_dram = []
    for hf in range(2):
        h2h = dram_pool.tile([hb * P, DH], bf16, name=f"h2an{hf}")
        h2half_dram.append(h2h)
        h2f = dram_pool.tile([NCORES * hb * P, DH], bf16,
                             addr_space="Shared", name=f"h2full{hf}")
        h2full_dram.append(h2f)

    for hf in range(2):
        h2_pm = h2half_dram[hf][:].rearrange("(p kl) d -> kl p d", kl=hb)
        for c in range(hb):
            cl = hf * hb + c
            hn_ps = tp_psum.tile([P, P], f32, name="hn_ps", tag="tp")
            nc.tensor.matmul(
                hn_ps[:], hT_sb[:, cl * P:(cl + 1) * P], ident[:],
                is_transpose=True, start=True, stop=True,
            )
            h2c_sb = fin_pool.tile([P, DH], bf16, name="h2c_sb")
            nc.scalar.activation(h2c_sb[:, 0:dout], hn_ps[:], AF.Copy,
                                 scale=v2loc_sb[:, cl:cl + 1])
            nc.scalar.activation(h2c_sb[:, dout:dout + 1],
                                 anhi_sb[:, cl:cl + 1], AF.Copy)
            nc.scalar.activation(h2c_sb[:, dout + 1:dout + 2],
                                 anlo_sb[:, cl:cl + 1], AF.Copy)
            nc.sync.dma_start(h2_pm[c], h2c_sb[:])
        nc.gpsimd.collective_compute(
            "AllGather", ALU.bypass, replica_groups=groups,
            ins=[h2half_dram[hf].opt()], outs=[h2full_dram[hf].opt()],
        )

    # ---- Phase 3: read gathered halves, derive w2/v2 columns ---------------
    h2big = []
    w2col = []
    v2col = []
    for hf in range(2):
        h2b = ph1_pool.tile([P, half_c * DH], bf16, name=f"h2big{hf}")
        for cb in range(NCORES):
            nc.sync.dma_start(
                h2b[:, cb * hb * DH:(cb + 1) * hb * DH],
                h2full_dram[hf][cb * hb * P:(cb + 1) * hb * P, :].rearrange(
                    "(p kl) d -> p (kl d)", kl=hb),
            )
        h2big.append(h2b)
        h3 = h2b[:].rearrange("p (c d) -> p c d", d=DH)
        anf = ph1_pool.tile([P, half_c], f32, name=f"anf{hf}")
        nc.vector.tensor_tensor(anf[:], h3[:, :, dout], h3[:, :, dout + 1],
                                ALU.add)
        w2 = ph1_pool.tile([P, half_c], f32, name=f"w2col{hf}")
        nc.scalar.activation(w2[:], anf[:], AF.Exp, scale=0.8)
        w2col.append(w2)
        v2 = ph1_pool.tile([P, half_c], bf16, name=f"v2col{hf}")
        nc.scalar.activation(v2[:], anf[:], AF.Exp, scale=0.2)
        v2col.append(v2)

    # invw_bcast[p, i] = exp(-0.8 * a_s_local[i]) for every partition p
    wrow_sb = ph1_pool.tile([1, s], bf16, name="wrow_sb")
    nc.scalar.activation(wrow_sb[:], av_sb[0:1, :], AF.Exp, scale=-0.8)
    ones_sb = const_pool.tile([1, P], bf16, name="ones_sb")
    nc.gpsimd.memset(ones_sb[:], 1.0)
    invw_sb = const_pool.tile([P, s], bf16, name="invw_sb")
    for b in range(s // nb1):
        wb_ps = tp_psum.tile([P, nb1], f32, name="wb_ps", tag="tp")
        nc.tensor.matmul(
            wb_ps[:], ones_sb[:], wrow_sb[:, b * nb1:(b + 1) * nb1],
            start=True, stop=True,
        )
        nc.scalar.activation(invw_sb[:, b * nb1:(b + 1) * nb1], wb_ps[:],
                             AF.Copy)

    # ---- Phase 4: main loop over adj blocks --------------------------------
    mm_ps = [acc_psum.tile([P, nb], f32, name=f"mm_ps{b}") for b in range(ib_n)]
    rs_ps = [acc_psum.tile([1, nb], f32, name=f"rs_ps{b}") for b in range(ib_n)]
    for pos in range(jc_n):
        hf, q32, r = pos // half_c, pos % half_c, pos % GP
        if r == 0:
            G = pos // GP
            adj_t = adj_pool.tile([P, GP * s], bf16, name="adj_t")
            nc.sync.dma_start(
                adj_t[:],
                adjt[G * GP * P:(G + 1) * GP * P, :].rearrange(
                    "(p r) i -> p (r i)", r=GP),
            )
        q_t = q_pool.tile([P, s], bf16, name="q_t")
        nc.vector.scalar_tensor_tensor(
            q_t[:], invw_sb[:], w2col[hf][:, q32:q32 + 1],
            adj_t[:, r * s:(r + 1) * s],
            op0=ALU.max, op1=ALU.mult,
        )
        for b in range(ib_n):
            nc.tensor.matmul(
                mm_ps[b][:], h2big[hf][:, q32 * DH:q32 * DH + dout],
                q_t[:, b * nb:(b + 1) * nb],
                start=(pos == 0), stop=(pos == jc_n - 1),
            )
        for b in range(ib_n):
            nc.tensor.matmul(
                rs_ps[b][:], v2col[hf][:, q32:q32 + 1],
                q_t[:, b * nb:(b + 1) * nb],
                start=(pos == 0), stop=(pos == jc_n - 1),
            )

    # ---- Phase 5: normalize, relu, transpose out ---------------------------
    rs_sb = ph1_pool.tile([1, s], f32, name="rs_sb")
    for b in range(ib_n):
        nc.scalar.activation(rs_sb[:, b * nb:(b + 1) * nb], rs_ps[b][:],
                             AF.Copy)
    rs_dram = dram_pool.tile([sc_n, P], f32, name="rs_dram")
    nc.sync.dma_start(rs_dram[:].rearrange("k p -> (k p)")[None, :], rs_sb[0:1, :])
    rs_raw = ph1_pool.tile([sc_n, P], f32, name="rs_raw")
    nc.sync.dma_start(rs_raw[:], rs_dram[:])
    rsT_ps = tp_psum.tile([P, sc_n], f32, name="rsT_ps", tag="tp")
    nc.tensor.matmul(rsT_ps[:], rs_raw[:], ident[:sc_n, :sc_n],
                     is_transpose=True, start=True, stop=True)
    rrT_sb = ph1_pool.tile([P, sc_n], f32, name="rrT_sb")
    nc.vector.reciprocal(rrT_sb[:], rsT_ps[:])

    mo_sb = ph1_pool.tile([P, s], f32, name="mo_sb")
    for b in range(ib_n):
        nc.scalar.activation(mo_sb[:, b * nb:(b + 1) * nb], mm_ps[b][:],
                             AF.Copy)
    for c in range(sc_n):
        ot_ps = tp_psum.tile([P, P], f32, name="ot_ps", tag="tp")
        nc.tensor.matmul(
            ot_ps[:], mo_sb[:, c * P:(c + 1) * P], ident[:],
            is_transpose=True, start=True, stop=True,
        )
        oc_sb = fin_pool.tile([P, dout], f32, name="oc_sb")
        nc.scalar.activation(oc_sb[:], ot_ps[:], AF.Relu,
                             scale=rrT_sb[:, c:c + 1])
        nc.sync.dma_start(out[c * P:(c + 1) * P, :], oc_sb[:])


def build_nc(n=N, s=S, din=DIN, dout=DOUT):
    from contextlib import ExitStack

    import concourse.bacc as bacc
    import concourse.tile as tile

    nc = bacc.Bacc(
        "TRN2",
        target_bir_lowering=False,
        debug=False,
        num_devices=NCORES,
    )
    with tile.TileContext(nc) as tc, ExitStack() as ctx:
        _emit(nc, tc, ctx, n, s, din, dout)
    nc.compile()
    return nc


def prep_adjt(adj_slab):
    """[s, n] adj row-slab -> transposed [n, s] bf16, reordered into 16
    blocks of 512 rows (two half-passes over 8 source cores) with GP-row
    interleave inside each block."""
    import ml_dtypes

    adjt = adj_slab.T  # [n, s]
    n, s = adjt.shape
    P = 128
    blocks = []
    for hf in range(2):
        for m in range(NCORES):
            blk = adjt[m * S + hf * 512:m * S + hf * 512 + 512, :]
            blk = blk.reshape(GP, P, s).transpose(1, 0, 2).reshape(GP * P, s)
            blocks.append(blk)
    adjt2 = np.concatenate(blocks, axis=0)
    return np.ascontiguousarray(adjt2.astype(ml_dtypes.bfloat16))


def make_in_maps(x, adj, W, attn_self, attn_neigh, s=S):
    att = np.concatenate([attn_self, attn_neigh], axis=1).astype(np.float32)
    in_maps = []
    for c in range(NCORES):
        sl = slice(c * s, (c + 1) * s)
        in_maps.append({
            "adjt": prep_adjt(adj[sl, :]),
            "xt": np.ascontiguousarray(x[sl, :].T),
            "wmat": np.ascontiguousarray(W),
            "att": att,
        })
    return in_maps


def kernel(x, adj, W, attn_self, attn_neigh):
    from concourse.bass_utils import run_bass_kernel_spmd

    x = np.asarray(x, dtype=np.float32)
    adj = np.asarray(adj, dtype=np.float32)
    W = np.asarray(W, dtype=np.float32)
    attn_self = np.asarray(attn_self, dtype=np.float32)
    attn_neigh = np.asarray(attn_neigh, dtype=np.float32)

    nc = build_nc()
    in_maps = make_in_maps(x, adj, W, attn_self, attn_neigh)
    res = run_bass_kernel_spmd(nc, in_maps, list(range(NCORES)))
    return np.concatenate([res.results[c]["out"] for c in range(NCORES)], axis=0)
